# revision 45
# baseline (speedup 1.0000x reference)
"""GAT layer on 8 trn2 NeuronCores (Bass/Tile).

Sharding: edges sorted by target node; each core owns a contiguous range of
V/8 target nodes and every edge pointing into it, so attention normalizers
and message sums are core-local (no all-reduce). Node features are projected
into per-core Q/V tables (full, replicated compute) and a core-local K table.

Per core:
  phase 1: TensorE computes Q|V rows (bf16, interleaved 512B records) for all
           nodes into a DRAM table, plus a duplicated-row K table
           (row 2n == row 2n+1 == K[n]) for the core's own target range.
  phase 2: per 128-target-node window: dma_gather QV[src] (table split at row
           32768 for the int16 gather index) and K via PAIR-PACKED 512B
           records: edges are paired host-side so a pair shares one K record
           (record 2t = [K[t]|K[t]], record 2t+1 = [K[t]|K[t+1]] via an
           overlapping elem_step=128 access pattern), halving K descriptors;
           VectorE: per-edge logit = sum(Q[src]*K[tgt]) per head, bias =
           leaky_relu(ew*We+be), attention exp, message scaling; one-hot
           matrices (is_equal against an iota row) drive TensorE scatter
           matmuls accumulating [message | attn_exp] into PSUM per window;
           then normalize by degree, W_o matmul, leaky_relu, DMA out.

Host does only data movement: sorting/sharding/pairing/padding, dtype casts,
index wrapping, output concatenation.
"""

import sys, types, math
import numpy as np

try:
    import antenv.axon_hooks  # noqa: F401
except Exception:
    import antenv  # noqa: F401
    _ah = types.ModuleType("antenv.axon_hooks")
    _ah.get_axon_ntff_profile_hook = lambda: None
    sys.modules["antenv.axon_hooks"] = _ah

import bass_rust
import concourse.bass as bass
import concourse.mybir as mybir
import concourse.tile as tile
from concourse import bacc
from concourse.masks import make_identity

P = 128
NCORES = 8
SPLIT = 32768
NEG_SLOPE = 0.2
BF = mybir.dt.bfloat16
F32 = mybir.dt.float32
I16 = mybir.dt.int16
AX = mybir.AxisListType
AF = mybir.ActivationFunctionType
OP = mybir.AluOpType


def _wrap_idx(pos_idx):
    """dma_gather idx layout: position i -> (partition i%16, col i//16),
    replicated across the 8 Q7 cores (128 partitions)."""
    n = len(pos_idx)
    n16 = (n + 15) // 16
    flat = np.zeros(n16 * 16, dtype=np.int16)
    flat[:n] = pos_idx
    w = flat.reshape(n16, 16).T.copy()
    return np.tile(w, (8, 1))


def _pair_edges(ws, wt, we_, base):
    """Greedy-pair edges (sorted by tgt) so each pair shares one K record.
    Pair (t,t) -> record 2*(t-base); pair (t,t+1) -> record 2*(t-base)+1.
    Unpairable leftovers become a pair with a dummy second slot.
    Returns per-pair krec plus per-slot (src, rel, ew) arrays (2 slots/pair)."""
    m = len(wt)
    krec, s_src, s_rel, s_ew = [], [], [], []
    i = 0
    while i < m:
        t0 = int(wt[i])
        if i + 1 < m and int(wt[i + 1]) <= t0 + 1:
            d = int(wt[i + 1]) - t0
            krec.append(2 * (t0 - base) + d)
            s_src.extend((ws[i], ws[i + 1]))
            s_rel.extend((wt[i], wt[i + 1]))
            s_ew.extend((we_[i], we_[i + 1]))
            i += 2
        else:
            krec.append(2 * (t0 - base))
            s_src.extend((ws[i], 0))
            s_rel.extend((wt[i], -10 ** 9))
            s_ew.extend((we_[i], 0.0))
            i += 1
    return (np.asarray(krec, dtype=np.int64),
            np.asarray(s_src, dtype=np.int64),
            np.asarray(s_rel, dtype=np.int64),
            np.asarray(s_ew, dtype=np.float64))


def _prep(h, edge_index, edge_weight):
    V, D = h.shape
    src = np.asarray(edge_index[0], dtype=np.int64)
    tgt = np.asarray(edge_index[1], dtype=np.int64)
    ew = np.asarray(edge_weight, dtype=np.float32)

    VPC = (V + NCORES - 1) // NCORES
    NW = (VPC + P - 1) // P
    VTPAD = ((V + 1023) // 1024) * 1024

    order = np.argsort(tgt, kind="stable")
    s_src, s_tgt, s_ew = src[order], tgt[order], ew[order]
    core_id = s_tgt // VPC
    win_id = (s_tgt % VPC) // P

    # paired[c][w][half] = (krec per pair, src/rel/ew per slot)
    paired = [[[None, None] for _ in range(NW)] for _ in range(NCORES)]
    raw = [[[None, None] for _ in range(NW)] for _ in range(NCORES)]
    for c in range(NCORES):
        m_c = core_id == c
        cs, ct, cw_, cwin = s_src[m_c], s_tgt[m_c], s_ew[m_c], win_id[m_c]
        # per-core rotated table position: own node range sits at col 0, so
        # the kernel reads K-projection inputs from hT at fixed offsets
        cs = (cs - c * VPC) % V
        for w in range(NW):
            m_w = cwin == w
            ws, wt, we_ = cs[m_w], ct[m_w], cw_[m_w]
            lo = ws < SPLIT
            base = c * VPC  # krec relative to the core's node range
            paired[c][w][0] = _pair_edges(ws[lo], wt[lo], we_[lo], base)
            paired[c][w][1] = _pair_edges(ws[~lo] - SPLIT, wt[~lo], we_[~lo], base)
            raw[c][w][0] = (ws[lo], wt[lo], we_[lo])
            raw[c][w][1] = (ws[~lo] - SPLIT, wt[~lo], we_[~lo])

    # per-half mode: paired K records unless pair-rounding costs extra chunks
    nkb = np.zeros((NW, 2), dtype=np.int64)
    nch = np.zeros((NW, 2), dtype=np.int64)
    kmode = np.zeros((NW, 2), dtype=np.int64)  # 1 = paired
    for w in range(NW):
        for hlf in range(2):
            mxp = max(len(paired[c][w][hlf][0]) for c in range(NCORES))
            mxe = max(len(paired[c][w][hlf][1]) // 2 * 2 -
                      np.count_nonzero(paired[c][w][hlf][3] == 0)
                      for c in range(NCORES))  # placeholder, fixed below
            mxe = max(len(raw[c][w][hlf][0]) for c in range(NCORES))
            pair_chunks = 2 * max(1, (mxp + P - 1) // P)
            edge_chunks = max(1, (mxe + P - 1) // P)
            if pair_chunks <= edge_chunks:
                kmode[w, hlf] = 1
                nkb[w, hlf] = pair_chunks // 2
                nch[w, hlf] = pair_chunks
            else:
                kmode[w, hlf] = 0
                nkb[w, hlf] = 0
                nch[w, hlf] = edge_chunks
    nex_qv = np.zeros((NW, 2), dtype=np.int64)
    nex_k = np.zeros((NW, 2), dtype=np.int64)
    for w in range(NW):
        for hlf in range(2):
            mxe = max(len(raw[c][w][hlf][0]) for c in range(NCORES))
            mxp = max(len(paired[c][w][hlf][0]) for c in range(NCORES))
            if kmode[w, hlf]:
                # pair ip sits at slots (2*(ip//P))*P+p and (2*(ip//P)+1)*P+p:
                # a partial last pair-block still occupies BOTH its chunks
                B, r = mxp // P, mxp % P
                last = (2 * B + 1) * P + r if r else 2 * B * P
                nex_qv[w, hlf] = min(int(nch[w, hlf]) * P,
                                     ((last + 15) // 16) * 16)
                nex_k[w, hlf] = min(int(nkb[w, hlf]) * P, ((mxp + 15) // 16) * 16)
            else:
                nex_qv[w, hlf] = min(int(nch[w, hlf]) * P, ((mxe + 15) // 16) * 16)
                nex_k[w, hlf] = nex_qv[w, hlf]
    nch_tot = int(nch.sum())
    # process big windows first: short dependency chains drain the pipeline
    worder = np.argsort(-(nch[:, 0] + nch[:, 1]), kind="stable")

    cores = []
    for c in range(NCORES):
        src16_cols, k16_cols = [], []
        tgtrel = np.full((P, nch_tot), -1000.0, dtype=np.float32)
        ew4 = np.zeros((P, 4 * nch_tot), dtype=np.float32)
        ccol = 0
        for w in worder:
            for hlf in range(2):
                ncwh = int(nch[w, hlf])
                if kmode[w, hlf]:
                    kr, ss, sr, se = paired[c][w][hlf]
                    npair = int(nkb[w, hlf]) * P
                    pk = np.zeros(npair, dtype=np.int64)
                    pk[: len(kr)] = kr
                    n_slots = 2 * npair
                    sl_src = np.zeros(n_slots, dtype=np.int64)
                    sl_rel = np.full(n_slots, -1e9, dtype=np.float64)
                    sl_ew = np.zeros(n_slots, dtype=np.float64)
                    sl_src[: len(ss)] = ss
                    sl_rel[: len(sr)] = sr - (c * VPC + w * P)
                    sl_ew[: len(se)] = se
                    # pair ip -> slots (chunk 2*(ip//128)+s, partition ip%128)
                    ip = np.arange(npair)
                    jpos = np.zeros(n_slots, dtype=np.int64)
                    jpos[0::2] = (2 * (ip // P)) * P + (ip % P)
                    jpos[1::2] = (2 * (ip // P) + 1) * P + (ip % P)
                    qv_idx = np.zeros(n_slots, dtype=np.int64)
                    qv_idx[jpos] = sl_src
                    rel_j = np.full(n_slots, -1e9, dtype=np.float64)
                    rel_j[jpos] = sl_rel
                    ew_j = np.zeros(n_slots, dtype=np.float64)
                    ew_j[jpos] = sl_ew
                else:
                    ws2, wt2, we2 = raw[c][w][hlf]
                    n_slots = ncwh * P
                    qv_idx = np.zeros(n_slots, dtype=np.int64)
                    qv_idx[: len(ws2)] = ws2
                    rel_j = np.full(n_slots, -1e9, dtype=np.float64)
                    rel_j[: len(wt2)] = wt2 - (c * VPC + w * P)
                    ew_j = np.zeros(n_slots, dtype=np.float64)
                    ew_j[: len(we2)] = we2
                    pk = np.zeros(n_slots, dtype=np.int64)
                    pk[: len(wt2)] = 2 * (wt2 - c * VPC)
                src16_cols.append(_wrap_idx(qv_idx))
                k16_cols.append(_wrap_idx(pk))
                for j in range(ncwh):
                    tgtrel[:, ccol + j] = rel_j[j * P : (j + 1) * P]
                    ew4[:, 4 * (ccol + j) : 4 * (ccol + j + 1)] = \
                        np.repeat(ew_j[j * P : (j + 1) * P, None], 4, axis=1)
                ccol += ncwh
        cores.append(
            dict(
                src16=np.ascontiguousarray(np.concatenate(src16_cols, axis=1)),
                k16=np.ascontiguousarray(np.concatenate(k16_cols, axis=1)),
                tgtrel=tgtrel,
                ew4=ew4,
            )
        )

    kidx_cols = 0
    for w in range(NW):
        for hlf in range(2):
            kidx_cols += int(nkb[w, hlf]) * 8 if kmode[w, hlf] else int(nch[w, hlf]) * 8
    meta = dict(
        V=V, D=D, VPC=VPC, NW=NW, VTPAD=VTPAD, nkb=nkb, nch=nch, kmode=kmode,
        nex_qv=nex_qv, nex_k=nex_k,
        nch_tot=nch_tot, idx_cols=nch_tot * 8, kidx_cols=kidx_cols,
        worder=worder,
    )
    return cores, meta


def _build(meta, has_bqkv, has_bo, has_be=True):
    V, D = meta["V"], meta["D"]
    VPC, NW, VTPAD = meta["VPC"], meta["NW"], meta["VTPAD"]
    nkb, nch, nch_tot = meta["nkb"], meta["nch"], meta["nch_tot"]
    KROWS = NW * P
    INV_S = 1.0 / math.sqrt(D // 4)
    IDXC = meta["idx_cols"]
    KIDXC = meta["kidx_cols"]

    nc = bacc.Bacc(None, target_bir_lowering=False)

    hT = nc.declare_dram_parameter("hT", [P, VTPAD], BF, isOutput=False)
    Wqv = nc.declare_dram_parameter("Wqv", [P, 2 * D], BF, isOutput=False)
    Wk = nc.declare_dram_parameter("Wk", [P, D], BF, isOutput=False)
    Wo = nc.declare_dram_parameter("Wo", [P, D], BF, isOutput=False)
    We_t = nc.declare_dram_parameter("We_t", [P, 256], F32, isOutput=False)
    be_t = nc.declare_dram_parameter("be_t", [P, 256], F32, isOutput=False)
    bqkv_t = nc.declare_dram_parameter("bqkv_t", [P, 3 * D], F32, isOutput=False)
    bo_t = nc.declare_dram_parameter("bo_t", [P, D], F32, isOutput=False)
    iota_b = nc.declare_dram_parameter("iota_b", [P, P], BF, isOutput=False)
    src16 = nc.declare_dram_parameter("src16", [P, IDXC], I16, isOutput=False)
    k16 = nc.declare_dram_parameter("k16", [P, KIDXC], I16, isOutput=False)
    tgtrel = nc.declare_dram_parameter("tgtrel", [P, nch_tot], F32, isOutput=False)
    ew4 = nc.declare_dram_parameter("ew4", [P, 4 * nch_tot], BF, isOutput=False)
    out = nc.declare_dram_parameter("out", [KROWS, D], F32, isOutput=True)

    QVtab = nc.dram_tensor("QVtab", [VTPAD, 2 * D], BF)
    Ktab2 = nc.dram_tensor("Ktab2", [2 * KROWS + 2, D], BF)
    NT = VTPAD // P
    KT = KROWS // P

    with tile.TileContext(nc) as tc:
        with tc.tile_pool(name="const", bufs=1) as constp:
            wqv_t = constp.tile([P, 2 * D], BF)
            wk_t = constp.tile([P, D], BF)
            wo_t = constp.tile([P, D], BF)
            we_c = constp.tile([P, 256], F32)
            be_c = constp.tile([P, 256], F32)
            iota_c = constp.tile([P, P], BF)
            ident = constp.tile([P, P], BF)
            k16_c = constp.tile([P, KIDXC], I16)
            tgtrel_f = constp.tile([P, nch_tot], F32)
            ew4_c = constp.tile([P, 4 * nch_tot], BF)
            nc.sync.dma_start(out=wqv_t[:], in_=Wqv[:])
            make_identity(nc, ident[:])
            bqkv_c = constp.tile([P, 3 * D], F32)
            bo_c = constp.tile([P, D], F32)

            # ---------------- phase 1: projection tables ----------------
            with (
                tc.tile_pool(name="hbuf", bufs=1) as hbufp,
                tc.tile_pool(name="p1ps", bufs=3, space="PSUM") as p1ps,
                tc.tile_pool(name="p1psk", bufs=2, space="PSUM") as p1psk,
                tc.tile_pool(name="p1sb", bufs=10) as p1sb,
            ):
                hT_s = hbufp.tile([P, VTPAD], BF)
                NSLAB = 32
                slab = VTPAD // NSLAB
                for s in range(NSLAB):
                    nc.sync.dma_start(out=hT_s[:, s * slab : (s + 1) * slab],
                                      in_=hT[:, s * slab : (s + 1) * slab])
                # phase-2 constants: loaded after the h slabs so they never
                # delay the phase-1 critical path
                nc.sync.dma_start(out=wk_t[:], in_=Wk[:])
                nc.sync.dma_start(out=wo_t[:], in_=Wo[:])
                nc.sync.dma_start(out=we_c[:], in_=We_t[:])
                nc.sync.dma_start(out=be_c[:], in_=be_t[:])
                nc.sync.dma_start(out=iota_c[:], in_=iota_b[:])
                nc.sync.dma_start(out=k16_c[:], in_=k16[:])
                nc.sync.dma_start(out=tgtrel_f[:], in_=tgtrel[:])
                nc.sync.dma_start(out=ew4_c[:], in_=ew4[:])
                if has_bqkv:
                    nc.sync.dma_start(out=bqkv_c[:], in_=bqkv_t[:])
                if has_bo:
                    nc.sync.dma_start(out=bo_c[:], in_=bo_t[:])
                QVtab_r = QVtab.rearrange("(g t p) d -> g p t d", t=4, p=P)
                # K table with duplicated rows: row 2n == row 2n+1 == K[n],
                # built in 4-window groups interleaved into the QV loop

                def k_group(g4, nt):
                    ps = p1psk.tile([P, 4, D], F32, tag="kps")
                    for tt in range(nt):
                        t = g4 * 4 + tt
                        lhs = hT_s[:, t * P : (t + 1) * P]
                        nc.tensor.matmul(out=ps[:, tt, :], lhsT=lhs, rhs=wk_t[:],
                                         start=True, stop=True)
                        if has_bqkv:
                            nc.vector.tensor_add(out=ps[:, tt, :],
                                                 in0=ps[:, tt, :],
                                                 in1=bqkv_c[:, D : 2 * D])
                    kv = p1sb.tile([P, 4, 2, D], BF, tag="ksb")
                    nc.scalar.copy(
                        out=kv[:, 0:nt],
                        in_=ps[:, 0:nt].unsqueeze(2).to_broadcast([P, nt, 2, D]))
                    dst = Ktab2[g4 * 1024 : g4 * 1024 + nt * 256, :].rearrange(
                        "(t p s) d -> p t s d", p=P, s=2)
                    nc.sync.dma_start(out=dst, in_=kv[:, 0:nt])

                for g in range(NT // 4):
                    stage = p1sb.tile([P, 4, 2 * D], BF, tag="qvsb")
                    ps = p1ps.tile([P, 4, 2 * D], F32, tag="qvps")
                    for tt in range(4):
                        t = g * 4 + tt
                        lhs = hT_s[:, t * P : (t + 1) * P]
                        nc.tensor.matmul(out=ps[:, tt, :], lhsT=lhs, rhs=wqv_t[:],
                                         start=True, stop=True)
                    if has_bqkv:
                        for tt in range(4):
                            nc.vector.tensor_add(out=ps[:, tt, 0:D],
                                                 in0=ps[:, tt, 0:D],
                                                 in1=bqkv_c[:, 0:D])
                            nc.vector.tensor_add(out=ps[:, tt, D : 2 * D],
                                                 in0=ps[:, tt, D : 2 * D],
                                                 in1=bqkv_c[:, 2 * D : 3 * D])
                    if g % 2 == 0:
                        nc.scalar.copy(out=stage[:], in_=ps[:])
                    else:
                        nc.vector.tensor_copy(out=stage[:], in_=ps[:])
                    nc.sync.dma_start(out=QVtab_r[g], in_=stage[:])
                    if g % 2 == 1 and g // 2 <= (KT - 1) // 4:
                        g4 = g // 2
                        k_group(g4, min(4, KT - g4 * 4))

            # ---------------- phase 2: edge processing ----------------
            with (
                tc.tile_pool(name="sidx", bufs=4) as sidxp,
                tc.tile_pool(name="gth", bufs=5) as gthp,
                tc.tile_pool(name="work", bufs=4) as workp,
                tc.tile_pool(name="small", bufs=6) as smallp,
                tc.tile_pool(name="oh", bufs=8) as ohp,
                tc.tile_pool(name="accps", bufs=3, space="PSUM") as accps,
                tc.tile_pool(name="tailps", bufs=2, space="PSUM") as tailps,
            ):
                # tables must land in DRAM before any gather reads them
                tc.strict_bb_all_engine_barrier()

                qv_lo = QVtab[0:SPLIT, :]
                qv_hi = QVtab[SPLIT:VTPAD, :]
                # K gather source: overlapping records, elem_step=128 elems
                k_src = bass_rust.AP(Ktab2[:].tensor, 0,
                                     [[D, 2 * KROWS], [1, 2 * D]])
                c0 = 0      # chunk column cursor
                i16 = 0     # qv idx column cursor
                ki16 = 0    # k idx column cursor
                kmode = meta["kmode"]
                nex_qv, nex_k = meta["nex_qv"], meta["nex_k"]
                for wi, w in enumerate(meta["worder"]):
                    nl, nh = int(nch[w, 0]), int(nch[w, 1])
                    ncw = nl + nh
                    warm = wi < 5  # pool depth: gather full so no slot is stale
                    eql = nl * P if warm else int(nex_qv[w, 0])
                    eqh = nh * P if warm else int(nex_qv[w, 1])
                    s16 = sidxp.tile([P, ncw * 8], I16, tag="s16")
                    nc.sync.dma_start(out=s16[:], in_=src16[:, i16 : i16 + ncw * 8])
                    qv_g = gthp.tile([P, ncw, 2 * D], BF, tag="qvg")
                    k_g = gthp.tile([P, ncw, D], BF, tag="kg")
                    nc.gpsimd.dma_gather(
                        out_ap=qv_g[:, 0 : (eql + P - 1) // P, :], in_ap=qv_lo,
                        idxs_ap=s16[:, 0 : nl * 8],
                        num_idxs=eql, num_idxs_reg=eql, elem_size=2 * D,
                        single_packet=False,
                    )
                    nc.gpsimd.dma_gather(
                        out_ap=qv_g[:, nl : nl + (eqh + P - 1) // P, :], in_ap=qv_hi,
                        idxs_ap=s16[:, nl * 8 : ncw * 8],
                        num_idxs=eqh, num_idxs_reg=eqh, elem_size=2 * D,
                        single_packet=False,
                    )
                    if kmode[w, 0] and kmode[w, 1]:
                        nb = int(nkb[w, 0]) + int(nkb[w, 1])
                        ek = nb * P if warm else \
                            int(nkb[w, 0]) * P + int(nex_k[w, 1])
                        nc.gpsimd.dma_gather(
                            out_ap=k_g[:, 0 : 2 * ((ek + P - 1) // P), :]
                            .rearrange("p (b s) d -> p b (s d)", s=2),
                            in_ap=k_src,
                            idxs_ap=k16_c[:, ki16 : ki16 + nb * 8],
                            num_idxs=ek, num_idxs_reg=ek,
                            elem_size=2 * D, elem_step=D,
                            single_packet=False,
                        )
                        ki16 += nb * 8
                        halves = []
                    else:
                        halves = [0, 1]
                    cbase = 0
                    for hlf in halves:
                        nc_h = int(nch[w, hlf])
                        if kmode[w, hlf]:
                            nb = int(nkb[w, hlf])
                            ek = nb * P if warm else int(nex_k[w, hlf])
                            nc.gpsimd.dma_gather(
                                out_ap=k_g[:, cbase : cbase + 2 * ((ek + P - 1) // P), :]
                                .rearrange("p (b s) d -> p b (s d)", s=2),
                                in_ap=k_src,
                                idxs_ap=k16_c[:, ki16 : ki16 + nb * 8],
                                num_idxs=ek, num_idxs_reg=ek,
                                elem_size=2 * D, elem_step=D,
                                single_packet=False,
                            )
                            ki16 += nb * 8
                        else:
                            ek = nc_h * P if warm else int(nex_k[w, hlf])
                            nc.gpsimd.dma_gather(
                                out_ap=k_g[:, cbase : cbase + (ek + P - 1) // P, :],
                                in_ap=Ktab2[:],
                                idxs_ap=k16_c[:, ki16 : ki16 + nc_h * 8],
                                num_idxs=ek, num_idxs_reg=ek,
                                elem_size=D,
                                single_packet=False,
                            )
                            ki16 += nc_h * 8
                        cbase += nc_h

                    qk = workp.tile([P, ncw, D], BF, tag="qk")
                    nc.vector.tensor_mul(out=qk[:], in0=qv_g[:, :, 0:D], in1=k_g[:])
                    qk4 = qk[:].rearrange("p c (h f) -> p c h f", f=D // 4)
                    t16 = workp.tile([P, ncw, 4, 16], BF, tag="t16")
                    nc.vector.tensor_add(out=t16[:], in0=qk4[:, :, :, 0:16],
                                         in1=qk4[:, :, :, 16:32])
                    t8 = smallp.tile([P, ncw, 4, 8], BF, tag="t8")
                    nc.vector.tensor_add(out=t8[:], in0=t16[:, :, :, 0:8],
                                         in1=t16[:, :, :, 8:16])
                    t4f = smallp.tile([P, ncw, 4, 4], BF, tag="t4f")
                    nc.vector.tensor_add(out=t4f[:], in0=t8[:, :, :, 0:4],
                                         in1=t8[:, :, :, 4:8])
                    t2f = smallp.tile([P, ncw, 4, 2], BF, tag="t2f")
                    nc.vector.tensor_add(out=t2f[:], in0=t4f[:, :, :, 0:2],
                                         in1=t4f[:, :, :, 2:4])
                    logits = smallp.tile([P, ncw, 4], BF, tag="logits")
                    nc.vector.tensor_add(
                        out=logits[:].rearrange("p c (h x) -> p c h x", x=1),
                        in0=t2f[:, :, :, 0:1], in1=t2f[:, :, :, 1:2])
                    ew_f = ew4_c[:, 4 * c0 : 4 * (c0 + ncw)]
                    biasp = smallp.tile([P, ncw * 4], F32, tag="biasp")
                    nc.vector.tensor_mul(out=biasp[:], in0=ew_f,
                                         in1=we_c[:, 0 : ncw * 4])
                    if has_be:
                        nc.vector.tensor_add(out=biasp[:], in0=biasp[:],
                                             in1=be_c[:, 0 : ncw * 4])
                    ebx = smallp.tile([P, ncw * 4], F32, tag="ebx")
                    nc.vector.scalar_tensor_tensor(
                        out=ebx[:], in0=biasp[:], scalar=NEG_SLOPE, in1=biasp[:],
                        op0=OP.mult, op1=OP.max)
                    nc.scalar.activation(out=ebx[:], in_=ebx[:], func=AF.Exp)
                    el = smallp.tile([P, ncw * 4], F32, tag="el")
                    nc.scalar.activation(
                        out=el[:],
                        in_=logits[:].rearrange("p c h -> p (c h)"),
                        func=AF.Exp, scale=INV_S,
                    )
                    attn_e = smallp.tile([P, ncw * 4], F32, tag="attne")
                    nc.vector.tensor_mul(out=attn_e[:], in0=el[:], in1=ebx[:])
                    attn_w = smallp.tile([P, ncw * 4], F32, tag="attnw")
                    nc.vector.tensor_mul(out=attn_w[:], in0=attn_e[:], in1=ew_f)
                    aw_x = workp.tile([P, ncw, D], BF, tag="awx")
                    nc.scalar.copy(
                        out=aw_x[:].rearrange("p c (h f) -> p c h f", f=D // 4),
                        in_=attn_w[:]
                        .rearrange("p (c h) -> p c h", h=4)
                        .to_broadcast([P, ncw, 4, D // 4]),
                    )
                    rhs = workp.tile([P, ncw, D + 4], BF, tag="rhs")
                    nc.vector.tensor_mul(out=rhs[:, :, 0:D], in0=aw_x[:],
                                         in1=qv_g[:, :, D : 2 * D])
                    nc.scalar.copy(
                        out=rhs[:, :, D : D + 4],
                        in_=attn_e[:].rearrange("p (c h) -> p c h", h=4),
                    )

                    acc = accps.tile([P, D + 4], F32, tag="acc")
                    for c in range(ncw):
                        oh = ohp.tile([P, P], BF, tag="oh")
                        nc.vector.tensor_scalar(
                            out=oh[:], in0=iota_c[:],
                            scalar1=tgtrel_f[:, c0 + c : c0 + c + 1], scalar2=None,
                            op0=OP.is_equal,
                        )
                        nc.tensor.matmul(out=acc[:], lhsT=oh[:], rhs=rhs[:, c, :],
                                         start=(c == 0), stop=(c == ncw - 1))

                    r4 = smallp.tile([P, 4], F32, tag="r4")
                    nc.vector.tensor_scalar_add(out=r4[:], in0=acc[:, D : D + 4],
                                                scalar1=1e-16)
                    nc.vector.reciprocal(out=r4[:], in_=r4[:])
                    mn = smallp.tile([P, D], BF, tag="mn")
                    F_h = D // 4
                    for hh in range(4):
                        nc.scalar.mul(out=mn[:, hh * F_h : (hh + 1) * F_h],
                                      in_=acc[:, hh * F_h : (hh + 1) * F_h],
                                      mul=r4[:, hh : hh + 1])
                    pst = tailps.tile([P, P], BF, tag="pst")
                    nc.tensor.transpose(out=pst[:], in_=mn[:], identity=ident[:])
                    mT = smallp.tile([P, P], BF, tag="mT")
                    nc.scalar.copy(out=mT[:], in_=pst[:])
                    pso = tailps.tile([P, D], F32, tag="pso")
                    nc.tensor.matmul(out=pso[:], lhsT=mT[:], rhs=wo_t[:],
                                     start=True, stop=True)
                    res = smallp.tile([P, D], F32, tag="res")
                    if has_bo:
                        res2 = smallp.tile([P, D], F32, tag="res2")
                        nc.vector.tensor_add(out=res[:], in0=pso[:], in1=bo_c[:])
                        nc.scalar.mul(out=res2[:], in_=res[:], mul=NEG_SLOPE)
                        nc.vector.tensor_tensor(out=res[:], in0=res[:], in1=res2[:],
                                                op=OP.max)
                    else:
                        res2 = smallp.tile([P, D], F32, tag="res2")
                        nc.scalar.mul(out=res2[:], in_=pso[:], mul=NEG_SLOPE)
                        nc.vector.tensor_tensor(out=res[:], in0=pso[:], in1=res2[:],
                                                op=OP.max)
                    nc.sync.dma_start(out=out[w * P : (w + 1) * P, :], in_=res[:])

                    c0 += ncw
                    i16 += ncw * 8

    nc.compile()
    return nc


def kernel(h, edge_index, edge_weight, Wq, bq, Wk, bk, Wv, bv, Wo, bo, We, be,
           _run=None):
    from concourse.bass_utils import run_bass_kernel_spmd

    h = np.asarray(h, dtype=np.float32)
    Wq, Wk, Wv, Wo = (np.asarray(x, dtype=np.float32) for x in (Wq, Wk, Wv, Wo))
    bq, bk, bv, bo = (np.asarray(x, dtype=np.float32) for x in (bq, bk, bv, bo))
    We = np.asarray(We, dtype=np.float32).reshape(1, 4)
    be = np.asarray(be, dtype=np.float32)
    V, D = h.shape

    cores, meta = _prep(h, edge_index, edge_weight)
    VPC, NW, VTPAD = meta["VPC"], meta["NW"], meta["VTPAD"]
    KROWS = NW * P

    has_bqkv = any(np.any(b != 0) for b in (bq, bk, bv))
    has_bo = bool(np.any(bo != 0))
    nc = _build(meta, has_bqkv, has_bo, has_be=bool(np.any(be != 0)))

    iota_np = np.tile(np.arange(P, dtype=np.float32)[None, :], (P, 1)).astype(
        mybir.dt.np(BF)
    )
    we_np = np.tile(We.reshape(1, 4), (P, 64)).astype(np.float32)
    be_np = np.tile(be.reshape(1, 4), (P, 64)).astype(np.float32)
    bqkv_np = np.tile(
        np.concatenate([bq, bk, bv]).reshape(1, 3 * D), (P, 1)
    ).astype(np.float32)
    bo_np = np.tile(bo.reshape(1, D), (P, 1)).astype(np.float32)
    common = dict(
        Wqv=np.concatenate([Wq, Wv], axis=1).astype(mybir.dt.np(BF)),
        Wk=Wk.astype(mybir.dt.np(BF)),
        Wo=Wo.astype(mybir.dt.np(BF)),
        We_t=we_np, be_t=be_np, bqkv_t=bqkv_np, bo_t=bo_np, iota_b=iota_np,
    )
    in_maps = []
    hbf = h.T.astype(mybir.dt.np(BF))
    for c in range(NCORES):
        hrot = np.zeros((P, VTPAD), dtype=mybir.dt.np(BF))
        r = c * VPC
        hrot[:, : V - r] = hbf[:, r:]
        hrot[:, V - r : V] = hbf[:, :r]
        m = dict(common)
        m.update(
            hT=hrot,
            src16=cores[c]["src16"],
            k16=cores[c]["k16"],
            tgtrel=cores[c]["tgtrel"],
            ew4=cores[c]["ew4"].astype(mybir.dt.np(BF)),
        )
        in_maps.append(m)

    if _run is not None:          # test hook (CoreSim etc.)
        return _run(nc, in_maps, meta)

    # the axon transport occasionally fails transiently; one retry is cheap
    try:
        res = run_bass_kernel_spmd(nc, in_maps, core_ids=list(range(NCORES)))
    except Exception:
        res = run_bass_kernel_spmd(nc, in_maps, core_ids=list(range(NCORES)))
    outs = [r["out"][:VPC] for r in res.results]
    return np.concatenate(outs, axis=0)[:V].astype(np.float32)
